# revision 42
# baseline (speedup 1.0000x reference)
"""GATv2Conv on 8 Trainium2 NeuronCores — edge-parallel, dst-sharded.

Strategy (per spec sharding_hint, edge parallelism variant):
  * Host sorts edges by dst and splits them into 8 contiguous dst-node
    ranges with ~equal edge counts.  Each core owns a disjoint set of
    destination nodes, so softmax stats and output aggregation are fully
    local — no collectives at all.
  * Each core (SPMD, one NEFF):
      Phase 1: xl = x @ W_l for ALL nodes (replicated compute),
               xr = x @ W_r for its own dst range; both stored as bf16
               rows in a DRAM scratch tensor `xcat`.
      Phase 2: edges are processed in "windows" of <=2048 edge slots
               whose dst span <128 nodes.  Per window:
                 - dma_gather pulls xl[src] rows (256B bf16) into SBUF.
                   The 4 fixed src-quarter regions go to 4 DIFFERENT
                   SWDGE queues (queue_num=0..3) so descriptor
                   generation runs on 4 Q7 core pairs concurrently
                   (a single-queue gather was the baseline bottleneck:
                   ~9ns/edge of serial Q7 time),
                 - per-window metadata (gather idx, fp8 one-hot S^T,
                   dst_rel) arrives in ONE combined DMA; the s4[e,t,n]
                   one-hot for aggregation is built on-device by a DVE
                   iota-compare,
                 - z^T per 512-edge super-tile: 4 PE transposes of the
                   gathered rows (start only on the first) + ONE wide
                   N=512 scatter matmul xr^T @ sT accumulated on top,
                 - scores: ScalarE Prelu(alpha=0.2) gives leaky(z^T) in
                   one op (Prelu honors alpha; Lrelu's is hardwired) and
                   shares the exp_and_others ACT table set with Exp, then
                   one N=4 matmul per strip against att; ScalarE exp -> p
                   written straight into the den columns of the u*p tile,
                 - aggregation: out[n,f] += s4^T @ (u * p), with den as
                   4 extra rhs columns,
                 - finalize: out = out/(den+eps) + bias (bf16), DMA the
                   128 node rows out.
  * Host concatenates the 8 per-core row ranges and fixes zero-degree
    rows to `bias` (softmax over an empty segment).

No max-subtraction is needed in the softmax: scores are O(+-10) and all
accumulation happens in fp32 PSUM, so exp() is safely in range.
"""

import numpy as np
import ml_dtypes

import concourse.bass as bass
import concourse.bacc as bacc
import concourse.mybir as mybir
import concourse.tile as tile
from concourse import library_config

BF16 = ml_dtypes.bfloat16
FP8 = ml_dtypes.float8_e4m3

H, C, F, D = 4, 32, 128, 128  # heads, channels/head, H*C, input dim
NCORES = 8
P = 128                       # partitions
EPW = 2048                    # edge slots per window (16 tiles of 128)
SLOTS = EPW // P              # 16
NSUP = EPW // 512             # 4 super-tiles (512 edges) per window
NQ = 4                        # src-quarter split (dma_gather idx is int16)
DUMMY_COL = 999.0             # dst_rel sentinel -> one-hot never matches
EPS = 1e-16
FP8_ONE = 0x38                # float8_e4m3 bit pattern of 1.0
NEG_SLOPE = 0.2
USE_LRELU = True              # ScalarE Prelu (1 ACT + 1 mm per strip)
#   vs the 0.6z+0.4|z| identity (2 ACT + 2 mm); Prelu is not implemented
#   in CoreSim, so _sim runs force it off unless the sim is patched.
TRIM_PAD = True               # pad unused gather slots with -1: the HW
#   ucode trims trailing negative idxs (skips their descriptors). CoreSim
#   asserts num_idxs_reg == count(idx>=0), so _sim runs force this off.


def _round_up(a, b):
    return (a + b - 1) // b * b


def _wrap16(flat, width):
    """int16 idx layout for dma_gather/scatter: value j at
    [j%16, j//16], replicated across the 8 Q7 core groups."""
    tmp = np.zeros(width * 16, np.int16)
    tmp[:len(flat)] = flat
    return np.tile(tmp.reshape(width, 16).T, (8, 1))


# ----------------------------------------------------------------- host prep


def _split_cores(sdst, n_nodes, n_edges):
    eb = [0]
    nb = [0]
    for c in range(1, NCORES):
        pos = min(n_edges - 1, (n_edges * c) // NCORES)
        node = int(sdst[pos])
        eb.append(int(np.searchsorted(sdst, node)))
        nb.append(node)
    eb.append(n_edges)
    nb.append(n_nodes)
    return eb, nb


QCAP = EPW // NQ  # 512 slots per fixed src-quarter region


def _build_core_windows(ssrc_c, sdst_c, nb_c, nr_c, qrows):
    """Pack one core's (dst-sorted) edges into fixed 2048-slot windows
    with four FIXED 512-slot src-quarter regions (slot layout is static,
    identical across cores).  A window closes when any quarter region is
    full or the dst span would reach 128 nodes."""
    ne = len(ssrc_c)
    if ne == 0:
        nodes = np.zeros(0, np.int64)
        counts = np.zeros(0, np.int64)
        seg_of_edge = np.zeros(0, np.int64)
    else:
        change = np.flatnonzero(np.diff(sdst_c)) + 1
        starts = np.concatenate(([0], change))
        nodes = sdst_c[starts]
        counts = np.diff(np.concatenate((starts, [ne])))
        seg_of_edge = np.repeat(np.arange(len(nodes)), counts)

    equarter = ssrc_c // qrows  # [ne]
    nseg = len(nodes)
    segq = np.zeros((nseg, NQ), np.int64)
    if ne:
        np.add.at(segq, (seg_of_edge, equarter), 1)
    assert nseg == 0 or segq.max() <= QCAP, "per-quarter degree too big"

    win_segs = []  # (first_seg, one_past_last_seg)
    i = 0
    while i < nseg:
        base = nodes[i]
        qc = np.zeros(NQ, np.int64)
        j = i
        while j < nseg and nodes[j] - base < P and (qc + segq[j]).max() <= QCAP:
            qc += segq[j]
            j += 1
        assert j > i, "single segment does not fit a window"
        win_segs.append((i, j))
        i = j
    wc = len(win_segs)

    # -1 = unused slot: dma_gather trims trailing negative idxs per call,
    # so padded tails of each quarter region cost no descriptor time.
    pad = -1 if TRIM_PAD else 0
    uidx = np.full((wc, P, SLOTS), pad, np.int16)
    dstrel = np.full((wc, P, SLOTS), DUMMY_COL, np.float32)
    win_nb = np.zeros(wc, np.int64)
    own_end = np.zeros(wc, np.int64)

    for w, (si, sj) in enumerate(win_segs):
        win_nb[w] = nodes[si]
        e0 = int(np.searchsorted(seg_of_edge, si))
        e1 = int(np.searchsorted(seg_of_edge, sj - 1, side="right"))
        es = ssrc_c[e0:e1]
        ed = sdst_c[e0:e1]
        eq = equarter[e0:e1]
        for q in range(NQ):
            sel = eq == q
            cq = int(sel.sum())
            if cq:
                slots = q * QCAP + np.arange(cq)
                pp = slots % P
                ss = slots // P
                uidx[w, pp, ss] = (es[sel] - q * qrows).astype(np.int16)
                dstrel[w, pp, ss] = (ed[sel] - win_nb[w]).astype(np.float32)
        own_end[w] = nodes[sj] if sj < nseg else nb_c + nr_c
        own_end[w] = min(own_end[w], win_nb[w] + P)

    return dict(win_nb=win_nb, own_end=own_end, uidx=uidx,
                dstrel=dstrel, wc=wc)


# ------------------------------------------------------------- bass program


def _build_nc(W, npad_xl, nrx):
    """Per-core SPMD bass program (fixed 4x512 quarter slot layout)."""
    nc = bacc.Bacc("TRN2", target_bir_lowering=False, debug=False,
                   num_swdge_queues=NQ, dynamic_dma_scratch_size=49152)
    bf = mybir.dt.bfloat16
    f32 = mybir.dt.float32
    f8 = mybir.dt.float8e4
    i16 = mybir.dt.int16
    u8 = mybir.dt.uint8
    qrows = npad_xl // NQ

    xT = nc.dram_tensor("xT", [P, npad_xl], bf, kind="ExternalInput")
    xrT = nc.dram_tensor("xrT", [P, nrx], bf, kind="ExternalInput")
    Wl = nc.dram_tensor("Wl", [P, F], bf, kind="ExternalInput")
    Wr = nc.dram_tensor("Wr", [P, F], bf, kind="ExternalInput")
    # attm[:, :H] = 0.6*att, attm[:, H:2H] = 0.4*att (leaky identity path),
    # attm[:, 2H:3H] = att (Lrelu path)
    attm = nc.dram_tensor("attm", [P, 3 * H], bf, kind="ExternalInput")
    bias_bc = nc.dram_tensor("bias_bc", [P, F], f32, kind="ExternalInput")
    # combined per-window metadata, one DMA per window:
    #   [0:256)        int16 u-gather idx (wrapped layout, 16 values/column)
    #   [256:2304)     one-hot dst matrix S^T (fp8, exact 0/1)
    #   [2304:2336)    per-slot dst_rel bf16 (DUMMY_COL for empty slots)
    UW = EPW // 16
    WMB = 2 * UW + EPW + 2 * SLOTS
    wmeta = nc.dram_tensor("wmeta", [W, P, WMB], mybir.dt.uint8,
                           kind="ExternalInput")

    outp = nc.dram_tensor("outp", [W * P, F], bf, kind="ExternalOutput")
    xcat = nc.dram_tensor("xcat", [npad_xl + nrx, F], bf, kind="Internal")

    ident_np = np.eye(P, dtype=np.float32).astype(BF16)
    ident_d = nc.inline_tensor(ident_np, name="ident")
    iota_np = np.tile(np.arange(P, dtype=np.float32), (P, SLOTS)).astype(BF16)
    iota_d = nc.inline_tensor(iota_np, name="iota16")

    Abs = mybir.ActivationFunctionType.Abs
    Exp = mybir.ActivationFunctionType.Exp
    # HW-probed: Lrelu's alpha is IGNORED (hardwired 0.01 slope); Prelu
    # honors alpha exactly, and parametric_relu shares the exp_and_others
    # ACT table set with Exp — no table reload between score and softmax.
    Prelu = mybir.ActivationFunctionType.Prelu

    with tile.TileContext(nc) as tc:
        with tc.tile_pool(name="const", bufs=1) as cpool:
            nc.gpsimd.load_library(library_config.mlp)
            ident_sb = cpool.tile([P, P], bf, tag="ident")
            nc.sync.dma_start(out=ident_sb[:], in_=ident_d.ap())
            iota_sb = cpool.tile([P, SLOTS, P], bf, tag="iota")
            nc.sync.dma_start(
                out=iota_sb[:],
                in_=iota_d.ap().rearrange("p (a b) -> p a b", a=SLOTS))
            attm_sb = cpool.tile([P, 3 * H], bf, tag="attm")
            nc.sync.dma_start(out=attm_sb[:], in_=attm[:])
            bias_sb = cpool.tile([P, F], f32, tag="bias")
            nc.sync.dma_start(out=bias_sb[:], in_=bias_bc[:])
            wl_sb = cpool.tile([P, F], bf, tag="wl")
            nc.sync.dma_start(out=wl_sb[:], in_=Wl[:])
            wr_sb = cpool.tile([P, F], bf, tag="wr")
            nc.sync.dma_start(out=wr_sb[:], in_=Wr[:])

            # ---------------- phase 1: xcat = [x @ Wl ; x_range @ Wr] (bf16)
            with (
                tc.tile_pool(name="ph1", bufs=3) as p1,
                tc.tile_pool(name="ph1ps", bufs=2, space="PSUM") as p1ps,
            ):
                CH = 2048

                def linear_chunks(src_T, w_sb, row0, nchunks):
                    for k in range(nchunks):
                        xt = p1.tile([P, CH], bf, tag="xt")
                        nc.sync.dma_start(
                            out=xt[:], in_=src_T[:, CH * k: CH * (k + 1)])
                        # matmul j computes nodes {16m+j}: output partition m
                        # holds node 16m+j, so partition m owns 16 CONSECUTIVE
                        # xcat rows -> 4KB-contiguous write descriptors
                        # (vs 16x256B with the plain strip order).
                        xtr = xt[:].rearrange("p (m j) -> p j m", j=16)
                        ps = p1ps.tile([P, CH], f32, tag="ps1")
                        for j in range(16):
                            nc.tensor.matmul(
                                out=ps[:, P * j: P * (j + 1)],
                                lhsT=xtr[:, j, :],
                                rhs=w_sb[:],
                                start=True, stop=True)
                        st = p1.tile([P, 16, F], bf, tag="st")
                        stv = st[:].rearrange("p a b -> p (a b)")
                        if k % 2 == 0:
                            nc.vector.tensor_copy(out=stv, in_=ps[:])
                        else:
                            nc.scalar.copy(out=stv, in_=ps[:])
                        nc.sync.dma_start(
                            out=xcat[row0 + CH * k: row0 + CH * (k + 1), :]
                            .rearrange("(p j) f -> p j f", p=P),
                            in_=st[:])

                # xr windows first: window w's score matmul needs xr_w, and
                # the per-quarter gathers only need their xl quarter — doing
                # xr first maximizes the chance of phase-1/phase-2 overlap.
                linear_chunks(xrT, wr_sb, npad_xl, nrx // CH)
                linear_chunks(xT, wl_sb, 0, npad_xl // CH)

            # ---------------- phase 2: edge windows
            with (
                tc.tile_pool(name="win", bufs=6) as wp,
                tc.tile_pool(name="gat", bufs=6) as gp,
                tc.tile_pool(name="mid", bufs=6) as mp,
                tc.tile_pool(name="fin", bufs=4) as fp,
                tc.tile_pool(name="pszt", bufs=3, space="PSUM") as ps_zt,
                tc.tile_pool(name="pssc", bufs=3, space="PSUM") as ps_sc,
                tc.tile_pool(name="psod", bufs=2, space="PSUM") as ps_od,
            ):
                for w in range(W):
                    # one combined metadata load per window:
                    # [gidx i16 (256B) | sT fp8 (2048B) | drel bf16 (32B)]
                    wm = wp.tile([P, WMB], u8, tag="wm")
                    nc.sync.dma_start(out=wm[:], in_=wmeta[w])
                    gx = wm[:, 0:2 * UW].bitcast(i16)
                    sT = wm[:, 2 * UW: 2 * UW + EPW].bitcast(f8)
                    dr = wm[:, 2 * UW + EPW:].bitcast(bf)
                    # fp8 one-hot: exact 0/1, half the DVE write traffic,
                    # and fp8 weight loads for the 16 agg matmuls
                    s4 = wp.tile([P, SLOTS, P], f8, tag="s4")
                    nc.vector.tensor_tensor(
                        out=s4[:], in0=iota_sb[:],
                        in1=dr[:, :, None].broadcast_to([P, SLOTS, P]),
                        op=mybir.AluOpType.is_equal)
                    xr_w = wp.tile([P, F], bf, tag="xr_w")
                    nc.sync.dma_start(
                        out=xr_w[:],
                        in_=xcat[npad_xl + P * w: npad_xl + P * (w + 1), :])

                    g = gp.tile([P, SLOTS, F], bf, tag="g")
                    if w < 6:
                        # first use of each ring buffer: clear so skipped
                        # (-1-trimmed) slots never hold inf/NaN bit patterns
                        nc.vector.memset(g[:], 0.0)
                    for q in range(NQ):
                        off = q * QCAP
                        nc.gpsimd.dma_gather(
                            g[:, off // P: (off + QCAP) // P, :],
                            xcat[q * qrows: (q + 1) * qrows, :],
                            gx[:, off // 16: (off + QCAP) // 16],
                            QCAP, QCAP, F,
                            queue_num=q)

                    # scores
                    pp = ps_sc.tile([P, SLOTS * H], f32, tag="pp")
                    for s in range(NSUP):
                        zt = ps_zt.tile([P, 512], f32, tag="zt")
                        for t in range(4):
                            e = 4 * s + t
                            nc.tensor.matmul(
                                out=zt[:, P * t: P * (t + 1)],
                                lhsT=g[:, e, :], rhs=ident_sb[:],
                                start=(t == 0), stop=False,
                                skip_group_check=True)
                        nc.tensor.matmul(
                            out=zt[:],
                            lhsT=xr_w[:],
                            rhs=sT[:, 512 * s: 512 * (s + 1)],
                            start=False, stop=True,
                            skip_group_check=True)
                        if USE_LRELU:
                            l_sb = mp.tile([P, 512], bf, tag="l_sb")
                            nc.scalar.activation(out=l_sb[:], in_=zt[:],
                                                 func=Prelu, alpha=NEG_SLOPE)
                            for t in range(4):
                                e = 4 * s + t
                                nc.tensor.matmul(
                                    out=pp[:, H * e: H * (e + 1)],
                                    lhsT=l_sb[:, P * t: P * (t + 1)],
                                    rhs=attm_sb[:, 2 * H: 3 * H],
                                    start=True, stop=True)
                        else:
                            z_sb = mp.tile([P, 512], bf, tag="z_sb")
                            nc.scalar.copy(out=z_sb[:], in_=zt[:])
                            a_sb = mp.tile([P, 512], bf, tag="a_sb")
                            nc.scalar.activation(out=a_sb[:], in_=zt[:],
                                                 func=Abs)
                            for t in range(4):
                                e = 4 * s + t
                                nc.tensor.matmul(
                                    out=pp[:, H * e: H * (e + 1)],
                                    lhsT=z_sb[:, P * t: P * (t + 1)],
                                    rhs=attm_sb[:, :H],
                                    start=True, stop=False)
                                nc.tensor.matmul(
                                    out=pp[:, H * e: H * (e + 1)],
                                    lhsT=a_sb[:, P * t: P * (t + 1)],
                                    rhs=attm_sb[:, H: 2 * H],
                                    start=False, stop=True)

                    # aggregation — den rides as 4 extra rhs columns; one
                    # window-wide exp writes p straight into those columns,
                    # the u*p multiply runs per super-tile so aggregation
                    # matmuls can start as soon as their slice is ready
                    xjw = mp.tile([P, SLOTS, F + H], bf, tag="xjw")
                    nc.scalar.activation(
                        out=xjw[:, :, F:],
                        in_=pp[:].rearrange("p (a b) -> p a b", b=H),
                        func=Exp)
                    nc.vector.tensor_tensor(
                        out=xjw[:, :, 0:F]
                        .rearrange("p t (h c) -> p t h c", h=H),
                        in0=g[:].rearrange("p t (h c) -> p t h c", h=H),
                        in1=xjw[:, :, F:][:, :, :, None]
                        .broadcast_to([P, SLOTS, H, C]),
                        op=mybir.AluOpType.mult)
                    pod = ps_od.tile([P, F + H], f32, tag="pod")
                    for e in range(SLOTS):
                        nc.tensor.matmul(
                            out=pod[:], lhsT=s4[:, e, :],
                            rhs=xjw[:, e, :],
                            start=(e == 0), stop=(e == SLOTS - 1))

                    dn = fp.tile([P, H], f32, tag="dn")
                    nc.vector.tensor_scalar_add(out=dn[:], in0=pod[:, F:],
                                                scalar1=EPS)
                    rd = fp.tile([P, H], f32, tag="rd")
                    nc.vector.reciprocal(out=rd[:], in_=dn[:])
                    fin = fp.tile([P, H, C], f32, tag="fin")
                    nc.vector.tensor_tensor(
                        out=fin[:],
                        in0=pod[:, 0:F].rearrange("p (h c) -> p h c", h=H),
                        in1=rd[:, :, None].broadcast_to([P, H, C]),
                        op=mybir.AluOpType.mult)
                    fin2 = fp.tile([P, F], bf, tag="fin2")
                    nc.vector.tensor_tensor(
                        out=fin2[:],
                        in0=fin[:].rearrange("p h c -> p (h c)"),
                        in1=bias_sb[:], op=mybir.AluOpType.add)
                    nc.sync.dma_start(
                        out=outp[P * w: P * (w + 1), :], in_=fin2[:])

    nc.compile()
    return nc


# ------------------------------------------------------------------- driver


def _prepare(x, edge_index, W_l, W_r, att, bias):
    n_nodes = x.shape[0]
    n_edges = edge_index.shape[1]
    src = np.asarray(edge_index[0], np.int64)
    dst = np.asarray(edge_index[1], np.int64)
    order = np.argsort(dst, kind="stable")
    ssrc = src[order]
    sdst = dst[order]

    eb, nb = _split_cores(sdst, n_nodes, n_edges)
    npad_xl = _round_up(n_nodes, 2048)  # divisible by 4 quarters of 512-mult
    qrows = npad_xl // NQ
    assert qrows <= 32768

    cores = []
    for c in range(NCORES):
        nr_c = nb[c + 1] - nb[c]
        assert nr_c < 32000
        cores.append(_build_core_windows(
            ssrc[eb[c]:eb[c + 1]], sdst[eb[c]:eb[c + 1]], nb[c], nr_c, qrows))

    W = max(cd["wc"] for cd in cores)
    nrx = _round_up(max(W * P, 2048), 2048)

    xb = np.asarray(x, np.float32).astype(BF16)
    xT_np = np.zeros((P, npad_xl), BF16)
    xT_np[:, :n_nodes] = xb.T

    wl_np = np.asarray(W_l, np.float32).astype(BF16)
    wr_np = np.asarray(W_r, np.float32).astype(BF16)
    att_np = np.asarray(att, np.float32)
    attm_np = np.zeros((P, 3 * H), np.float32)
    for h in range(H):
        attm_np[C * h:C * (h + 1), h] = 0.6 * att_np[h]
        attm_np[C * h:C * (h + 1), H + h] = 0.4 * att_np[h]
        attm_np[C * h:C * (h + 1), 2 * H + h] = att_np[h]
    attm_np = attm_np.astype(BF16)
    bias_np = np.asarray(bias, np.float32)
    bias_bc_np = np.tile(bias_np[None, :], (P, 1)).astype(np.float32)

    iota128 = np.arange(P, dtype=np.float32)

    in_maps = []
    for c in range(NCORES):
        cd = cores[c]
        wc = cd["wc"]

        UW = EPW // 16
        WMB = 2 * UW + EPW + 2 * SLOTS
        gidx_np = np.full((W, P, UW), -1 if TRIM_PAD else 0, np.int16)
        # one-hot dst matrix S^T as uint8 bit patterns, viewed fp8
        sT_np = np.zeros((W, P, EPW), np.uint8)
        drel_np = np.full((W, P, SLOTS), DUMMY_COL, np.float32)

        for w in range(wc):
            # uidx is [P, SLOTS] slot-shaped (slot j at [j%128, j//128]);
            # rewrap for the int16 idx plane (value j at [j%16, j//16]).
            uflat = cd["uidx"][w].T.reshape(-1)  # slot order: s*128+p
            gidx_np[w] = _wrap16(uflat, UW)
            dr = cd["dstrel"][w]                 # [P, SLOTS] float
            drel_np[w] = dr
            drf = dr.T.reshape(-1)               # flat slot-major
            sT_np[w] = (iota128[:, None] == drf[None, :]) * np.uint8(FP8_ONE)

        wmeta_np = np.empty((W, P, WMB), np.uint8)
        wmeta_np[:, :, 0:2 * UW] = gidx_np.view(np.uint8)
        wmeta_np[:, :, 2 * UW: 2 * UW + EPW] = sT_np
        wmeta_np[:, :, 2 * UW + EPW:] = \
            drel_np.astype(BF16).view(np.uint8)

        # window-major xr source: column 128*w + j = x[win_nb[w] + j]
        xrT_np = np.zeros((P, nrx), BF16)
        for w in range(wc):
            lo = int(cd["win_nb"][w])
            hi = min(lo + P, n_nodes)
            xrT_np[:, P * w: P * w + (hi - lo)] = xb[lo:hi].T

        in_maps.append({
            "xT": xT_np,
            "xrT": xrT_np,
            "Wl": wl_np,
            "Wr": wr_np,
            "attm": attm_np,
            "bias_bc": bias_bc_np,
            "wmeta": wmeta_np,
        })

    meta = dict(W=W, npad_xl=npad_xl, nrx=nrx, nb=nb,
                n_nodes=n_nodes, bias=bias_np, cores=cores,
                deg=np.bincount(dst, minlength=n_nodes))
    return in_maps, meta


_last_results = None


def kernel(x, edge_index, W_l, W_r, att, bias, _sim=False, _trace=False):
    global _last_results
    in_maps, meta = _prepare(x, edge_index, W_l, W_r, att, bias)
    nc = _build_nc(meta["W"], meta["npad_xl"], meta["nrx"])

    if _sim:
        from concourse.bass_interp import CoreSim
        results = []
        for c in range(NCORES):
            sim = CoreSim(nc, trace=False)
            for k, v in in_maps[c].items():
                sim.tensor(k)[:] = v
            sim.tensor("outp")[:] = 0.0
            sim.simulate()
            results.append({"outp": np.array(sim.tensor("outp"))})
    else:
        from concourse import bass_utils
        r = bass_utils.run_bass_kernel_spmd(
            nc, in_maps, core_ids=list(range(NCORES)), trace=_trace)
        _last_results = r
        results = r.results

    n_nodes = meta["n_nodes"]
    out = np.empty((n_nodes, F), np.float32)
    for c in range(NCORES):
        cd = meta["cores"][c]
        st = results[c]["outp"]
        for w in range(cd["wc"]):
            lo = int(cd["win_nb"][w])
            hi = int(cd["own_end"][w])
            out[lo:hi] = st[P * w: P * w + (hi - lo)]
    out[meta["deg"] == 0] = meta["bias"][None, :]
    return out


# revision 43
# speedup vs baseline: 1.1538x; 1.1538x over previous
"""GATv2Conv on 8 Trainium2 NeuronCores — edge-parallel, dst-sharded.

Strategy (per spec sharding_hint, edge parallelism variant):
  * Host sorts edges by dst and splits them into 8 contiguous dst-node
    ranges with ~equal edge counts.  Each core owns a disjoint set of
    destination nodes, so softmax stats and output aggregation are fully
    local — no collectives at all.
  * Each core (SPMD, one NEFF):
      Phase 1: xl = x @ W_l for ALL nodes (replicated compute),
               xr = x @ W_r for its own dst range; both stored as bf16
               rows in a DRAM scratch tensor `xcat`.
      Phase 2: edges are processed in "windows" of <=2048 edge slots
               whose dst span <128 nodes.  Per window:
                 - dma_gather pulls xl[src] rows (256B bf16) into SBUF.
                   The 4 fixed src-quarter regions go to 4 DIFFERENT
                   SWDGE queues (queue_num=0..3) so descriptor
                   generation runs on 4 Q7 core pairs concurrently
                   (a single-queue gather was the baseline bottleneck:
                   ~9ns/edge of serial Q7 time),
                 - per-window metadata (gather idx, fp8 one-hot S^T,
                   dst_rel) arrives in ONE combined DMA; the s4[e,t,n]
                   one-hot for aggregation is built on-device by a DVE
                   iota-compare,
                 - z^T per 512-edge super-tile: 4 PE transposes of the
                   gathered rows (start only on the first) + ONE wide
                   N=512 scatter matmul xr^T @ sT accumulated on top,
                 - scores: ScalarE Prelu(alpha=0.2) gives leaky(z^T) in
                   one op (Prelu honors alpha; Lrelu's is hardwired) and
                   shares the exp_and_others ACT table set with Exp, then
                   one N=4 matmul per strip against att; ScalarE exp -> p
                   written straight into the den columns of the u*p tile,
                 - aggregation: out[n,f] += s4^T @ (u * p), with den as
                   4 extra rhs columns,
                 - finalize: out = out/(den+eps) + bias (bf16), DMA the
                   128 node rows out.
  * Host concatenates the 8 per-core row ranges and fixes zero-degree
    rows to `bias` (softmax over an empty segment).

No max-subtraction is needed in the softmax: scores are O(+-10) and all
accumulation happens in fp32 PSUM, so exp() is safely in range.
"""

import numpy as np
import ml_dtypes

import concourse.bass as bass
import concourse.bacc as bacc
import concourse.mybir as mybir
import concourse.tile as tile
from concourse import library_config

BF16 = ml_dtypes.bfloat16
FP8 = ml_dtypes.float8_e4m3

H, C, F, D = 4, 32, 128, 128  # heads, channels/head, H*C, input dim
NCORES = 8
P = 128                       # partitions
EPW = 2048                    # edge slots per window (16 tiles of 128)
SLOTS = EPW // P              # 16
NSUP = EPW // 512             # 4 super-tiles (512 edges) per window
NQ = 4                        # src-quarter split (dma_gather idx is int16)
DUMMY_COL = 999.0             # dst_rel sentinel -> one-hot never matches
EPS = 1e-16
FP8_ONE = 0x38                # float8_e4m3 bit pattern of 1.0
NEG_SLOPE = 0.2
USE_LRELU = True              # ScalarE Prelu (1 ACT + 1 mm per strip)
#   vs the 0.6z+0.4|z| identity (2 ACT + 2 mm); Prelu is not implemented
#   in CoreSim, so _sim runs force it off unless the sim is patched.
TRIM_PAD = True               # pad unused gather slots with -1: the HW
#   ucode trims trailing negative idxs (skips their descriptors). CoreSim
#   asserts num_idxs_reg == count(idx>=0), so _sim runs force this off.


def _round_up(a, b):
    return (a + b - 1) // b * b


def _wrap16(flat, width):
    """int16 idx layout for dma_gather/scatter: value j at
    [j%16, j//16], replicated across the 8 Q7 core groups."""
    tmp = np.zeros(width * 16, np.int16)
    tmp[:len(flat)] = flat
    return np.tile(tmp.reshape(width, 16).T, (8, 1))


# ----------------------------------------------------------------- host prep


def _split_cores(sdst, n_nodes, n_edges):
    eb = [0]
    nb = [0]
    for c in range(1, NCORES):
        pos = min(n_edges - 1, (n_edges * c) // NCORES)
        node = int(sdst[pos])
        eb.append(int(np.searchsorted(sdst, node)))
        nb.append(node)
    eb.append(n_edges)
    nb.append(n_nodes)
    return eb, nb


QCAP = EPW // NQ  # 512 slots per fixed src-quarter region


def _build_core_windows(ssrc_c, sdst_c, nb_c, nr_c, qrows):
    """Pack one core's (dst-sorted) edges into fixed 2048-slot windows
    with four FIXED 512-slot src-quarter regions (slot layout is static,
    identical across cores).  A window closes when any quarter region is
    full or the dst span would reach 128 nodes."""
    ne = len(ssrc_c)
    if ne == 0:
        nodes = np.zeros(0, np.int64)
        counts = np.zeros(0, np.int64)
        seg_of_edge = np.zeros(0, np.int64)
    else:
        change = np.flatnonzero(np.diff(sdst_c)) + 1
        starts = np.concatenate(([0], change))
        nodes = sdst_c[starts]
        counts = np.diff(np.concatenate((starts, [ne])))
        seg_of_edge = np.repeat(np.arange(len(nodes)), counts)

    equarter = ssrc_c // qrows  # [ne]
    nseg = len(nodes)
    segq = np.zeros((nseg, NQ), np.int64)
    if ne:
        np.add.at(segq, (seg_of_edge, equarter), 1)
    assert nseg == 0 or segq.max() <= QCAP, "per-quarter degree too big"

    win_segs = []  # (first_seg, one_past_last_seg)
    i = 0
    while i < nseg:
        base = nodes[i]
        qc = np.zeros(NQ, np.int64)
        j = i
        while j < nseg and nodes[j] - base < P and (qc + segq[j]).max() <= QCAP:
            qc += segq[j]
            j += 1
        assert j > i, "single segment does not fit a window"
        win_segs.append((i, j))
        i = j
    wc = len(win_segs)

    # -1 = unused slot: dma_gather trims trailing negative idxs per call,
    # so padded tails of each quarter region cost no descriptor time.
    pad = -1 if TRIM_PAD else 0
    uidx = np.full((wc, P, SLOTS), pad, np.int16)
    dstrel = np.full((wc, P, SLOTS), DUMMY_COL, np.float32)
    win_nb = np.zeros(wc, np.int64)
    own_end = np.zeros(wc, np.int64)

    for w, (si, sj) in enumerate(win_segs):
        win_nb[w] = nodes[si]
        e0 = int(np.searchsorted(seg_of_edge, si))
        e1 = int(np.searchsorted(seg_of_edge, sj - 1, side="right"))
        es = ssrc_c[e0:e1]
        ed = sdst_c[e0:e1]
        eq = equarter[e0:e1]
        for q in range(NQ):
            sel = eq == q
            cq = int(sel.sum())
            if cq:
                slots = q * QCAP + np.arange(cq)
                pp = slots % P
                ss = slots // P
                uidx[w, pp, ss] = (es[sel] - q * qrows).astype(np.int16)
                dstrel[w, pp, ss] = (ed[sel] - win_nb[w]).astype(np.float32)
        own_end[w] = nodes[sj] if sj < nseg else nb_c + nr_c
        own_end[w] = min(own_end[w], win_nb[w] + P)

    return dict(win_nb=win_nb, own_end=own_end, uidx=uidx,
                dstrel=dstrel, wc=wc)


# ------------------------------------------------------------- bass program


def _build_nc(W, npad_xl, nrx):
    """Per-core SPMD bass program (fixed 4x512 quarter slot layout)."""
    nc = bacc.Bacc("TRN2", target_bir_lowering=False, debug=False,
                   num_swdge_queues=NQ, dynamic_dma_scratch_size=32768)
    bf = mybir.dt.bfloat16
    f32 = mybir.dt.float32
    f8 = mybir.dt.float8e4
    i16 = mybir.dt.int16
    u8 = mybir.dt.uint8
    qrows = npad_xl // NQ

    xT = nc.dram_tensor("xT", [P, npad_xl], bf, kind="ExternalInput")
    xrT = nc.dram_tensor("xrT", [P, nrx], bf, kind="ExternalInput")
    Wl = nc.dram_tensor("Wl", [P, F], bf, kind="ExternalInput")
    Wr = nc.dram_tensor("Wr", [P, F], bf, kind="ExternalInput")
    # attm[:, :H] = 0.6*att, attm[:, H:2H] = 0.4*att (leaky identity path),
    # attm[:, 2H:3H] = att (Lrelu path)
    attm = nc.dram_tensor("attm", [P, 3 * H], bf, kind="ExternalInput")
    bias_bc = nc.dram_tensor("bias_bc", [P, F], f32, kind="ExternalInput")
    # combined per-window metadata, one DMA per window:
    #   [0:256)        int16 u-gather idx (wrapped layout, 16 values/column)
    #   [256:2304)     one-hot dst matrix S^T (fp8, exact 0/1)
    #   [2304:2336)    per-slot dst_rel bf16 (DUMMY_COL for empty slots)
    UW = EPW // 16
    WMB = 2 * UW + EPW + 2 * SLOTS
    wmeta = nc.dram_tensor("wmeta", [W, P, WMB], mybir.dt.uint8,
                           kind="ExternalInput")

    outp = nc.dram_tensor("outp", [W * P, F], bf, kind="ExternalOutput")
    xcat = nc.dram_tensor("xcat", [npad_xl + nrx, F], bf, kind="Internal")

    ident_np = np.eye(P, dtype=np.float32).astype(BF16)
    ident_d = nc.inline_tensor(ident_np, name="ident")
    iota_np = np.tile(np.arange(P, dtype=np.float32), (P, SLOTS)).astype(BF16)
    iota_d = nc.inline_tensor(iota_np, name="iota16")

    Abs = mybir.ActivationFunctionType.Abs
    Exp = mybir.ActivationFunctionType.Exp
    # HW-probed: Lrelu's alpha is IGNORED (hardwired 0.01 slope); Prelu
    # honors alpha exactly, and parametric_relu shares the exp_and_others
    # ACT table set with Exp — no table reload between score and softmax.
    Prelu = mybir.ActivationFunctionType.Prelu

    with tile.TileContext(nc) as tc:
        with tc.tile_pool(name="const", bufs=1) as cpool:
            nc.gpsimd.load_library(library_config.mlp)
            ident_sb = cpool.tile([P, P], bf, tag="ident")
            nc.sync.dma_start(out=ident_sb[:], in_=ident_d.ap())
            iota_sb = cpool.tile([P, SLOTS, P], bf, tag="iota")
            nc.sync.dma_start(
                out=iota_sb[:],
                in_=iota_d.ap().rearrange("p (a b) -> p a b", a=SLOTS))
            attm_sb = cpool.tile([P, 3 * H], bf, tag="attm")
            nc.sync.dma_start(out=attm_sb[:], in_=attm[:])
            bias_sb = cpool.tile([P, F], f32, tag="bias")
            nc.sync.dma_start(out=bias_sb[:], in_=bias_bc[:])
            wl_sb = cpool.tile([P, F], bf, tag="wl")
            nc.sync.dma_start(out=wl_sb[:], in_=Wl[:])
            wr_sb = cpool.tile([P, F], bf, tag="wr")
            nc.sync.dma_start(out=wr_sb[:], in_=Wr[:])

            # ---------------- phase 1: xcat = [x @ Wl ; x_range @ Wr] (bf16)
            with (
                tc.tile_pool(name="ph1", bufs=3) as p1,
                tc.tile_pool(name="ph1ps", bufs=2, space="PSUM") as p1ps,
            ):
                CH = 2048

                def linear_chunks(src_T, w_sb, row0, nchunks):
                    for k in range(nchunks):
                        xt = p1.tile([P, CH], bf, tag="xt")
                        nc.sync.dma_start(
                            out=xt[:], in_=src_T[:, CH * k: CH * (k + 1)])
                        # matmul j computes nodes {16m+j}: output partition m
                        # holds node 16m+j, so partition m owns 16 CONSECUTIVE
                        # xcat rows -> 4KB-contiguous write descriptors
                        # (vs 16x256B with the plain strip order).
                        xtr = xt[:].rearrange("p (m j) -> p j m", j=16)
                        ps = p1ps.tile([P, CH], f32, tag="ps1")
                        for j in range(16):
                            nc.tensor.matmul(
                                out=ps[:, P * j: P * (j + 1)],
                                lhsT=xtr[:, j, :],
                                rhs=w_sb[:],
                                start=True, stop=True)
                        st = p1.tile([P, 16, F], bf, tag="st")
                        stv = st[:].rearrange("p a b -> p (a b)")
                        if k % 2 == 0:
                            nc.vector.tensor_copy(out=stv, in_=ps[:])
                        else:
                            nc.scalar.copy(out=stv, in_=ps[:])
                        nc.sync.dma_start(
                            out=xcat[row0 + CH * k: row0 + CH * (k + 1), :]
                            .rearrange("(p j) f -> p j f", p=P),
                            in_=st[:])

                # xr windows first: window w's score matmul needs xr_w, and
                # the per-quarter gathers only need their xl quarter — doing
                # xr first maximizes the chance of phase-1/phase-2 overlap.
                linear_chunks(xrT, wr_sb, npad_xl, nrx // CH)
                linear_chunks(xT, wl_sb, 0, npad_xl // CH)

            # ---------------- phase 2: edge windows
            with (
                tc.tile_pool(name="win", bufs=5) as wp,
                tc.tile_pool(name="gat", bufs=6) as gp,
                tc.tile_pool(name="mid", bufs=6) as mp,
                tc.tile_pool(name="fin", bufs=4) as fp,
                tc.tile_pool(name="pszt", bufs=3, space="PSUM") as ps_zt,
                tc.tile_pool(name="pssc", bufs=3, space="PSUM") as ps_sc,
                tc.tile_pool(name="psod", bufs=2, space="PSUM") as ps_od,
            ):
                for w in range(W):
                    # one combined metadata load per window:
                    # [gidx i16 (256B) | sT fp8 (2048B) | drel bf16 (32B)]
                    wm = wp.tile([P, WMB], u8, tag="wm")
                    nc.sync.dma_start(out=wm[:], in_=wmeta[w])
                    gx = wm[:, 0:2 * UW].bitcast(i16)
                    sT = wm[:, 2 * UW: 2 * UW + EPW].bitcast(f8)
                    dr = wm[:, 2 * UW + EPW:].bitcast(bf)
                    s4 = wp.tile([P, SLOTS, P], bf, tag="s4")
                    nc.vector.tensor_tensor(
                        out=s4[:], in0=iota_sb[:],
                        in1=dr[:, :, None].broadcast_to([P, SLOTS, P]),
                        op=mybir.AluOpType.is_equal)
                    xr_w = wp.tile([P, F], bf, tag="xr_w")
                    nc.sync.dma_start(
                        out=xr_w[:],
                        in_=xcat[npad_xl + P * w: npad_xl + P * (w + 1), :])

                    g = gp.tile([P, SLOTS, F], bf, tag="g")
                    if w < 6:
                        # first use of each ring buffer: clear so skipped
                        # (-1-trimmed) slots never hold inf/NaN bit patterns
                        nc.vector.memset(g[:], 0.0)
                    for q in range(NQ):
                        off = q * QCAP
                        nc.gpsimd.dma_gather(
                            g[:, off // P: (off + QCAP) // P, :],
                            xcat[q * qrows: (q + 1) * qrows, :],
                            gx[:, off // 16: (off + QCAP) // 16],
                            QCAP, QCAP, F,
                            queue_num=q)

                    # scores
                    pp = ps_sc.tile([P, SLOTS * H], f32, tag="pp")
                    for s in range(NSUP):
                        zt = ps_zt.tile([P, 512], f32, tag="zt")
                        for t in range(4):
                            e = 4 * s + t
                            nc.tensor.matmul(
                                out=zt[:, P * t: P * (t + 1)],
                                lhsT=g[:, e, :], rhs=ident_sb[:],
                                start=(t == 0), stop=False,
                                skip_group_check=True)
                        nc.tensor.matmul(
                            out=zt[:],
                            lhsT=xr_w[:],
                            rhs=sT[:, 512 * s: 512 * (s + 1)],
                            start=False, stop=True,
                            skip_group_check=True)
                        if USE_LRELU:
                            l_sb = mp.tile([P, 512], bf, tag="l_sb")
                            nc.scalar.activation(out=l_sb[:], in_=zt[:],
                                                 func=Prelu, alpha=NEG_SLOPE)
                            for t in range(4):
                                e = 4 * s + t
                                nc.tensor.matmul(
                                    out=pp[:, H * e: H * (e + 1)],
                                    lhsT=l_sb[:, P * t: P * (t + 1)],
                                    rhs=attm_sb[:, 2 * H: 3 * H],
                                    start=True, stop=True)
                        else:
                            z_sb = mp.tile([P, 512], bf, tag="z_sb")
                            nc.scalar.copy(out=z_sb[:], in_=zt[:])
                            a_sb = mp.tile([P, 512], bf, tag="a_sb")
                            nc.scalar.activation(out=a_sb[:], in_=zt[:],
                                                 func=Abs)
                            for t in range(4):
                                e = 4 * s + t
                                nc.tensor.matmul(
                                    out=pp[:, H * e: H * (e + 1)],
                                    lhsT=z_sb[:, P * t: P * (t + 1)],
                                    rhs=attm_sb[:, :H],
                                    start=True, stop=False)
                                nc.tensor.matmul(
                                    out=pp[:, H * e: H * (e + 1)],
                                    lhsT=a_sb[:, P * t: P * (t + 1)],
                                    rhs=attm_sb[:, H: 2 * H],
                                    start=False, stop=True)

                    # aggregation — den rides as 4 extra rhs columns; one
                    # window-wide exp writes p straight into those columns,
                    # the u*p multiply runs per super-tile so aggregation
                    # matmuls can start as soon as their slice is ready
                    xjw = mp.tile([P, SLOTS, F + H], bf, tag="xjw")
                    nc.scalar.activation(
                        out=xjw[:, :, F:],
                        in_=pp[:].rearrange("p (a b) -> p a b", b=H),
                        func=Exp)
                    nc.vector.tensor_tensor(
                        out=xjw[:, :, 0:F]
                        .rearrange("p t (h c) -> p t h c", h=H),
                        in0=g[:].rearrange("p t (h c) -> p t h c", h=H),
                        in1=xjw[:, :, F:][:, :, :, None]
                        .broadcast_to([P, SLOTS, H, C]),
                        op=mybir.AluOpType.mult)
                    pod = ps_od.tile([P, F + H], f32, tag="pod")
                    for e in range(SLOTS):
                        nc.tensor.matmul(
                            out=pod[:], lhsT=s4[:, e, :],
                            rhs=xjw[:, e, :],
                            start=(e == 0), stop=(e == SLOTS - 1))

                    dn = fp.tile([P, H], f32, tag="dn")
                    nc.vector.tensor_scalar_add(out=dn[:], in0=pod[:, F:],
                                                scalar1=EPS)
                    rd = fp.tile([P, H], f32, tag="rd")
                    nc.vector.reciprocal(out=rd[:], in_=dn[:])
                    fin = fp.tile([P, H, C], f32, tag="fin")
                    nc.vector.tensor_tensor(
                        out=fin[:],
                        in0=pod[:, 0:F].rearrange("p (h c) -> p h c", h=H),
                        in1=rd[:, :, None].broadcast_to([P, H, C]),
                        op=mybir.AluOpType.mult)
                    fin2 = fp.tile([P, F], bf, tag="fin2")
                    nc.vector.tensor_tensor(
                        out=fin2[:],
                        in0=fin[:].rearrange("p h c -> p (h c)"),
                        in1=bias_sb[:], op=mybir.AluOpType.add)
                    nc.sync.dma_start(
                        out=outp[P * w: P * (w + 1), :], in_=fin2[:])

    nc.compile()
    return nc


# ------------------------------------------------------------------- driver


def _prepare(x, edge_index, W_l, W_r, att, bias):
    n_nodes = x.shape[0]
    n_edges = edge_index.shape[1]
    src = np.asarray(edge_index[0], np.int64)
    dst = np.asarray(edge_index[1], np.int64)
    order = np.argsort(dst, kind="stable")
    ssrc = src[order]
    sdst = dst[order]

    eb, nb = _split_cores(sdst, n_nodes, n_edges)
    npad_xl = _round_up(n_nodes, 2048)  # divisible by 4 quarters of 512-mult
    qrows = npad_xl // NQ
    assert qrows <= 32768

    cores = []
    for c in range(NCORES):
        nr_c = nb[c + 1] - nb[c]
        assert nr_c < 32000
        cores.append(_build_core_windows(
            ssrc[eb[c]:eb[c + 1]], sdst[eb[c]:eb[c + 1]], nb[c], nr_c, qrows))

    W = max(cd["wc"] for cd in cores)
    nrx = _round_up(max(W * P, 2048), 2048)

    xb = np.asarray(x, np.float32).astype(BF16)
    xT_np = np.zeros((P, npad_xl), BF16)
    xT_np[:, :n_nodes] = xb.T

    wl_np = np.asarray(W_l, np.float32).astype(BF16)
    wr_np = np.asarray(W_r, np.float32).astype(BF16)
    att_np = np.asarray(att, np.float32)
    attm_np = np.zeros((P, 3 * H), np.float32)
    for h in range(H):
        attm_np[C * h:C * (h + 1), h] = 0.6 * att_np[h]
        attm_np[C * h:C * (h + 1), H + h] = 0.4 * att_np[h]
        attm_np[C * h:C * (h + 1), 2 * H + h] = att_np[h]
    attm_np = attm_np.astype(BF16)
    bias_np = np.asarray(bias, np.float32)
    bias_bc_np = np.tile(bias_np[None, :], (P, 1)).astype(np.float32)

    iota128 = np.arange(P, dtype=np.float32)

    in_maps = []
    for c in range(NCORES):
        cd = cores[c]
        wc = cd["wc"]

        UW = EPW // 16
        WMB = 2 * UW + EPW + 2 * SLOTS
        gidx_np = np.full((W, P, UW), -1 if TRIM_PAD else 0, np.int16)
        # one-hot dst matrix S^T as uint8 bit patterns, viewed fp8
        sT_np = np.zeros((W, P, EPW), np.uint8)
        drel_np = np.full((W, P, SLOTS), DUMMY_COL, np.float32)

        for w in range(wc):
            # uidx is [P, SLOTS] slot-shaped (slot j at [j%128, j//128]);
            # rewrap for the int16 idx plane (value j at [j%16, j//16]).
            uflat = cd["uidx"][w].T.reshape(-1)  # slot order: s*128+p
            gidx_np[w] = _wrap16(uflat, UW)
            dr = cd["dstrel"][w]                 # [P, SLOTS] float
            drel_np[w] = dr
            drf = dr.T.reshape(-1)               # flat slot-major
            sT_np[w] = (iota128[:, None] == drf[None, :]) * np.uint8(FP8_ONE)

        wmeta_np = np.empty((W, P, WMB), np.uint8)
        wmeta_np[:, :, 0:2 * UW] = gidx_np.view(np.uint8)
        wmeta_np[:, :, 2 * UW: 2 * UW + EPW] = sT_np
        wmeta_np[:, :, 2 * UW + EPW:] = \
            drel_np.astype(BF16).view(np.uint8)

        # window-major xr source: column 128*w + j = x[win_nb[w] + j]
        xrT_np = np.zeros((P, nrx), BF16)
        for w in range(wc):
            lo = int(cd["win_nb"][w])
            hi = min(lo + P, n_nodes)
            xrT_np[:, P * w: P * w + (hi - lo)] = xb[lo:hi].T

        in_maps.append({
            "xT": xT_np,
            "xrT": xrT_np,
            "Wl": wl_np,
            "Wr": wr_np,
            "attm": attm_np,
            "bias_bc": bias_bc_np,
            "wmeta": wmeta_np,
        })

    meta = dict(W=W, npad_xl=npad_xl, nrx=nrx, nb=nb,
                n_nodes=n_nodes, bias=bias_np, cores=cores,
                deg=np.bincount(dst, minlength=n_nodes))
    return in_maps, meta


_last_results = None


def kernel(x, edge_index, W_l, W_r, att, bias, _sim=False, _trace=False):
    global _last_results
    in_maps, meta = _prepare(x, edge_index, W_l, W_r, att, bias)
    nc = _build_nc(meta["W"], meta["npad_xl"], meta["nrx"])

    if _sim:
        from concourse.bass_interp import CoreSim
        results = []
        for c in range(NCORES):
            sim = CoreSim(nc, trace=False)
            for k, v in in_maps[c].items():
                sim.tensor(k)[:] = v
            sim.tensor("outp")[:] = 0.0
            sim.simulate()
            results.append({"outp": np.array(sim.tensor("outp"))})
    else:
        from concourse import bass_utils
        r = bass_utils.run_bass_kernel_spmd(
            nc, in_maps, core_ids=list(range(NCORES)), trace=_trace)
        _last_results = r
        results = r.results

    n_nodes = meta["n_nodes"]
    out = np.empty((n_nodes, F), np.float32)
    for c in range(NCORES):
        cd = meta["cores"][c]
        st = results[c]["outp"]
        for w in range(cd["wc"]):
            lo = int(cd["win_nb"][w])
            hi = int(cd["own_end"][w])
            out[lo:hi] = st[P * w: P * w + (hi - lo)]
    out[meta["deg"] == 0] = meta["bias"][None, :]
    return out


# revision 44
# speedup vs baseline: 1.1907x; 1.0320x over previous
"""GATv2Conv on 8 Trainium2 NeuronCores — edge-parallel, dst-sharded.

Strategy (per spec sharding_hint, edge parallelism variant):
  * Host sorts edges by dst and splits them into 8 contiguous dst-node
    ranges with ~equal edge counts.  Each core owns a disjoint set of
    destination nodes, so softmax stats and output aggregation are fully
    local — no collectives at all.
  * Each core (SPMD, one NEFF):
      Phase 1: xl = x @ W_l for ALL nodes (replicated compute),
               xr = x @ W_r for its own dst range; both stored as bf16
               rows in a DRAM scratch tensor `xcat`.
      Phase 2: edges are processed in "windows" of <=2048 edge slots
               whose dst span <128 nodes.  Per window:
                 - dma_gather pulls xl[src] rows (256B bf16) into SBUF.
                   The 4 fixed src-quarter regions go to 4 DIFFERENT
                   SWDGE queues (queue_num=0..3) so descriptor
                   generation runs on 4 Q7 core pairs concurrently
                   (a single-queue gather was the baseline bottleneck:
                   ~9ns/edge of serial Q7 time),
                 - per-window metadata (gather idx, fp8 one-hot S^T,
                   dst_rel) arrives in ONE combined DMA; the s4[e,t,n]
                   one-hot for aggregation is built on-device by a DVE
                   iota-compare,
                 - z^T per 512-edge super-tile: 4 PE transposes of the
                   gathered rows (start only on the first) + ONE wide
                   N=512 scatter matmul xr^T @ sT accumulated on top,
                 - scores: ScalarE Prelu(alpha=0.2) gives leaky(z^T) in
                   one op (Prelu honors alpha; Lrelu's is hardwired) and
                   shares the exp_and_others ACT table set with Exp, then
                   one N=4 matmul per strip against att; ScalarE exp -> p
                   written straight into the den columns of the u*p tile,
                 - aggregation: out[n,f] += s4^T @ (u * p), with den as
                   4 extra rhs columns,
                 - finalize: out = out/(den+eps) + bias (bf16), DMA the
                   128 node rows out.
  * Host concatenates the 8 per-core row ranges and fixes zero-degree
    rows to `bias` (softmax over an empty segment).

No max-subtraction is needed in the softmax: scores are O(+-10) and all
accumulation happens in fp32 PSUM, so exp() is safely in range.
"""

import numpy as np
import ml_dtypes

import concourse.bass as bass
import concourse.bacc as bacc
import concourse.mybir as mybir
import concourse.tile as tile
from concourse import library_config

BF16 = ml_dtypes.bfloat16
FP8 = ml_dtypes.float8_e4m3

H, C, F, D = 4, 32, 128, 128  # heads, channels/head, H*C, input dim
NCORES = 8
P = 128                       # partitions
EPW = 2048                    # edge slots per window (16 tiles of 128)
SLOTS = EPW // P              # 16
NSUP = EPW // 512             # 4 super-tiles (512 edges) per window
NQ = 4                        # src-quarter split (dma_gather idx is int16)
DUMMY_COL = 999.0             # dst_rel sentinel -> one-hot never matches
EPS = 1e-16
FP8_ONE = 0x38                # float8_e4m3 bit pattern of 1.0
NEG_SLOPE = 0.2
USE_LRELU = True              # ScalarE Prelu (1 ACT + 1 mm per strip)
#   vs the 0.6z+0.4|z| identity (2 ACT + 2 mm); Prelu is not implemented
#   in CoreSim, so _sim runs force it off unless the sim is patched.
TRIM_PAD = True               # pad unused gather slots with -1: the HW
#   ucode trims trailing negative idxs (skips their descriptors). CoreSim
#   asserts num_idxs_reg == count(idx>=0), so _sim runs force this off.


def _round_up(a, b):
    return (a + b - 1) // b * b


def _wrap16(flat, width):
    """int16 idx layout for dma_gather/scatter: value j at
    [j%16, j//16], replicated across the 8 Q7 core groups."""
    tmp = np.zeros(width * 16, np.int16)
    tmp[:len(flat)] = flat
    return np.tile(tmp.reshape(width, 16).T, (8, 1))


# ----------------------------------------------------------------- host prep


def _split_cores(sdst, n_nodes, n_edges):
    eb = [0]
    nb = [0]
    for c in range(1, NCORES):
        pos = min(n_edges - 1, (n_edges * c) // NCORES)
        node = int(sdst[pos])
        eb.append(int(np.searchsorted(sdst, node)))
        nb.append(node)
    eb.append(n_edges)
    nb.append(n_nodes)
    return eb, nb


QCAP = EPW // NQ  # 512 slots per fixed src-quarter region


def _build_core_windows(ssrc_c, sdst_c, nb_c, nr_c, qrows):
    """Pack one core's (dst-sorted) edges into fixed 2048-slot windows
    with four FIXED 512-slot src-quarter regions (slot layout is static,
    identical across cores).  A window closes when any quarter region is
    full or the dst span would reach 128 nodes."""
    ne = len(ssrc_c)
    if ne == 0:
        nodes = np.zeros(0, np.int64)
        counts = np.zeros(0, np.int64)
        seg_of_edge = np.zeros(0, np.int64)
    else:
        change = np.flatnonzero(np.diff(sdst_c)) + 1
        starts = np.concatenate(([0], change))
        nodes = sdst_c[starts]
        counts = np.diff(np.concatenate((starts, [ne])))
        seg_of_edge = np.repeat(np.arange(len(nodes)), counts)

    equarter = ssrc_c // qrows  # [ne]
    nseg = len(nodes)
    segq = np.zeros((nseg, NQ), np.int64)
    if ne:
        np.add.at(segq, (seg_of_edge, equarter), 1)
    assert nseg == 0 or segq.max() <= QCAP, "per-quarter degree too big"

    win_segs = []  # (first_seg, one_past_last_seg)
    i = 0
    while i < nseg:
        base = nodes[i]
        qc = np.zeros(NQ, np.int64)
        j = i
        while j < nseg and nodes[j] - base < P and (qc + segq[j]).max() <= QCAP:
            qc += segq[j]
            j += 1
        assert j > i, "single segment does not fit a window"
        win_segs.append((i, j))
        i = j
    wc = len(win_segs)

    # -1 = unused slot: dma_gather trims trailing negative idxs per call,
    # so padded tails of each quarter region cost no descriptor time.
    pad = -1 if TRIM_PAD else 0
    uidx = np.full((wc, P, SLOTS), pad, np.int16)
    dstrel = np.full((wc, P, SLOTS), DUMMY_COL, np.float32)
    win_nb = np.zeros(wc, np.int64)
    own_end = np.zeros(wc, np.int64)

    for w, (si, sj) in enumerate(win_segs):
        win_nb[w] = nodes[si]
        e0 = int(np.searchsorted(seg_of_edge, si))
        e1 = int(np.searchsorted(seg_of_edge, sj - 1, side="right"))
        es = ssrc_c[e0:e1]
        ed = sdst_c[e0:e1]
        eq = equarter[e0:e1]
        for q in range(NQ):
            sel = eq == q
            cq = int(sel.sum())
            if cq:
                # ascending src order within the quarter: the gather's
                # descriptor stream then reads HBM in ascending address
                # order (DRAM row-buffer locality) instead of randomly
                so = np.argsort(es[sel], kind="stable")
                esq = es[sel][so]
                edq = ed[sel][so]
                slots = q * QCAP + np.arange(cq)
                pp = slots % P
                ss = slots // P
                uidx[w, pp, ss] = (esq - q * qrows).astype(np.int16)
                dstrel[w, pp, ss] = (edq - win_nb[w]).astype(np.float32)
        own_end[w] = nodes[sj] if sj < nseg else nb_c + nr_c
        own_end[w] = min(own_end[w], win_nb[w] + P)

    return dict(win_nb=win_nb, own_end=own_end, uidx=uidx,
                dstrel=dstrel, wc=wc)


# ------------------------------------------------------------- bass program


def _build_nc(W, npad_xl, nrx):
    """Per-core SPMD bass program (fixed 4x512 quarter slot layout)."""
    nc = bacc.Bacc("TRN2", target_bir_lowering=False, debug=False,
                   num_swdge_queues=NQ, dynamic_dma_scratch_size=32768)
    bf = mybir.dt.bfloat16
    f32 = mybir.dt.float32
    f8 = mybir.dt.float8e4
    i16 = mybir.dt.int16
    u8 = mybir.dt.uint8
    qrows = npad_xl // NQ

    xT = nc.dram_tensor("xT", [P, npad_xl], bf, kind="ExternalInput")
    xrT = nc.dram_tensor("xrT", [P, nrx], bf, kind="ExternalInput")
    Wl = nc.dram_tensor("Wl", [P, F], bf, kind="ExternalInput")
    Wr = nc.dram_tensor("Wr", [P, F], bf, kind="ExternalInput")
    # attm[:, :H] = 0.6*att, attm[:, H:2H] = 0.4*att (leaky identity path),
    # attm[:, 2H:3H] = att (Lrelu path)
    attm = nc.dram_tensor("attm", [P, 3 * H], bf, kind="ExternalInput")
    bias_bc = nc.dram_tensor("bias_bc", [P, F], f32, kind="ExternalInput")
    # combined per-window metadata, one DMA per window:
    #   [0:256)        int16 u-gather idx (wrapped layout, 16 values/column)
    #   [256:2304)     one-hot dst matrix S^T (fp8, exact 0/1)
    #   [2304:2336)    per-slot dst_rel bf16 (DUMMY_COL for empty slots)
    UW = EPW // 16
    WMB = 2 * UW + EPW + 2 * SLOTS
    wmeta = nc.dram_tensor("wmeta", [W, P, WMB], mybir.dt.uint8,
                           kind="ExternalInput")

    outp = nc.dram_tensor("outp", [W * P, F], bf, kind="ExternalOutput")
    xcat = nc.dram_tensor("xcat", [npad_xl + nrx, F], bf, kind="Internal")

    ident_np = np.eye(P, dtype=np.float32).astype(BF16)
    ident_d = nc.inline_tensor(ident_np, name="ident")
    iota_np = np.tile(np.arange(P, dtype=np.float32), (P, SLOTS)).astype(BF16)
    iota_d = nc.inline_tensor(iota_np, name="iota16")

    Abs = mybir.ActivationFunctionType.Abs
    Exp = mybir.ActivationFunctionType.Exp
    # HW-probed: Lrelu's alpha is IGNORED (hardwired 0.01 slope); Prelu
    # honors alpha exactly, and parametric_relu shares the exp_and_others
    # ACT table set with Exp — no table reload between score and softmax.
    Prelu = mybir.ActivationFunctionType.Prelu

    with tile.TileContext(nc) as tc:
        with tc.tile_pool(name="const", bufs=1) as cpool:
            nc.gpsimd.load_library(library_config.mlp)
            ident_sb = cpool.tile([P, P], bf, tag="ident")
            nc.sync.dma_start(out=ident_sb[:], in_=ident_d.ap())
            iota_sb = cpool.tile([P, SLOTS, P], bf, tag="iota")
            nc.sync.dma_start(
                out=iota_sb[:],
                in_=iota_d.ap().rearrange("p (a b) -> p a b", a=SLOTS))
            attm_sb = cpool.tile([P, 3 * H], bf, tag="attm")
            nc.sync.dma_start(out=attm_sb[:], in_=attm[:])
            bias_sb = cpool.tile([P, F], f32, tag="bias")
            nc.sync.dma_start(out=bias_sb[:], in_=bias_bc[:])
            wl_sb = cpool.tile([P, F], bf, tag="wl")
            nc.sync.dma_start(out=wl_sb[:], in_=Wl[:])
            wr_sb = cpool.tile([P, F], bf, tag="wr")
            nc.sync.dma_start(out=wr_sb[:], in_=Wr[:])

            # ---------------- phase 1: xcat = [x @ Wl ; x_range @ Wr] (bf16)
            with (
                tc.tile_pool(name="ph1", bufs=3) as p1,
                tc.tile_pool(name="ph1ps", bufs=2, space="PSUM") as p1ps,
            ):
                CH = 2048

                def linear_chunks(src_T, w_sb, row0, nchunks):
                    for k in range(nchunks):
                        xt = p1.tile([P, CH], bf, tag="xt")
                        nc.sync.dma_start(
                            out=xt[:], in_=src_T[:, CH * k: CH * (k + 1)])
                        # matmul j computes nodes {16m+j}: output partition m
                        # holds node 16m+j, so partition m owns 16 CONSECUTIVE
                        # xcat rows -> 4KB-contiguous write descriptors
                        # (vs 16x256B with the plain strip order).
                        xtr = xt[:].rearrange("p (m j) -> p j m", j=16)
                        ps = p1ps.tile([P, CH], f32, tag="ps1")
                        for j in range(16):
                            nc.tensor.matmul(
                                out=ps[:, P * j: P * (j + 1)],
                                lhsT=xtr[:, j, :],
                                rhs=w_sb[:],
                                start=True, stop=True)
                        st = p1.tile([P, 16, F], bf, tag="st")
                        stv = st[:].rearrange("p a b -> p (a b)")
                        if k % 2 == 0:
                            nc.vector.tensor_copy(out=stv, in_=ps[:])
                        else:
                            nc.scalar.copy(out=stv, in_=ps[:])
                        nc.sync.dma_start(
                            out=xcat[row0 + CH * k: row0 + CH * (k + 1), :]
                            .rearrange("(p j) f -> p j f", p=P),
                            in_=st[:])

                # xr windows first: window w's score matmul needs xr_w, and
                # the per-quarter gathers only need their xl quarter — doing
                # xr first maximizes the chance of phase-1/phase-2 overlap.
                linear_chunks(xrT, wr_sb, npad_xl, nrx // CH)
                linear_chunks(xT, wl_sb, 0, npad_xl // CH)

            # ---------------- phase 2: edge windows
            with (
                tc.tile_pool(name="win", bufs=5) as wp,
                tc.tile_pool(name="gat", bufs=6) as gp,
                tc.tile_pool(name="mid", bufs=6) as mp,
                tc.tile_pool(name="fin", bufs=4) as fp,
                tc.tile_pool(name="pszt", bufs=3, space="PSUM") as ps_zt,
                tc.tile_pool(name="pssc", bufs=3, space="PSUM") as ps_sc,
                tc.tile_pool(name="psod", bufs=2, space="PSUM") as ps_od,
            ):
                for w in range(W):
                    # one combined metadata load per window:
                    # [gidx i16 (256B) | sT fp8 (2048B) | drel bf16 (32B)]
                    wm = wp.tile([P, WMB], u8, tag="wm")
                    nc.sync.dma_start(out=wm[:], in_=wmeta[w])
                    gx = wm[:, 0:2 * UW].bitcast(i16)
                    sT = wm[:, 2 * UW: 2 * UW + EPW].bitcast(f8)
                    dr = wm[:, 2 * UW + EPW:].bitcast(bf)
                    s4 = wp.tile([P, SLOTS, P], bf, tag="s4")
                    nc.vector.tensor_tensor(
                        out=s4[:], in0=iota_sb[:],
                        in1=dr[:, :, None].broadcast_to([P, SLOTS, P]),
                        op=mybir.AluOpType.is_equal)
                    xr_w = wp.tile([P, F], bf, tag="xr_w")
                    nc.sync.dma_start(
                        out=xr_w[:],
                        in_=xcat[npad_xl + P * w: npad_xl + P * (w + 1), :])

                    g = gp.tile([P, SLOTS, F], bf, tag="g")
                    if w < 6:
                        # first use of each ring buffer: clear so skipped
                        # (-1-trimmed) slots never hold inf/NaN bit patterns
                        nc.vector.memset(g[:], 0.0)
                    for q in range(NQ):
                        off = q * QCAP
                        nc.gpsimd.dma_gather(
                            g[:, off // P: (off + QCAP) // P, :],
                            xcat[q * qrows: (q + 1) * qrows, :],
                            gx[:, off // 16: (off + QCAP) // 16],
                            QCAP, QCAP, F,
                            queue_num=q)

                    # scores
                    pp = ps_sc.tile([P, SLOTS * H], f32, tag="pp")
                    for s in range(NSUP):
                        zt = ps_zt.tile([P, 512], f32, tag="zt")
                        for t in range(4):
                            e = 4 * s + t
                            nc.tensor.matmul(
                                out=zt[:, P * t: P * (t + 1)],
                                lhsT=g[:, e, :], rhs=ident_sb[:],
                                start=(t == 0), stop=False,
                                skip_group_check=True)
                        nc.tensor.matmul(
                            out=zt[:],
                            lhsT=xr_w[:],
                            rhs=sT[:, 512 * s: 512 * (s + 1)],
                            start=False, stop=True,
                            skip_group_check=True)
                        if USE_LRELU:
                            l_sb = mp.tile([P, 512], bf, tag="l_sb")
                            nc.scalar.activation(out=l_sb[:], in_=zt[:],
                                                 func=Prelu, alpha=NEG_SLOPE)
                            for t in range(4):
                                e = 4 * s + t
                                nc.tensor.matmul(
                                    out=pp[:, H * e: H * (e + 1)],
                                    lhsT=l_sb[:, P * t: P * (t + 1)],
                                    rhs=attm_sb[:, 2 * H: 3 * H],
                                    start=True, stop=True)
                        else:
                            z_sb = mp.tile([P, 512], bf, tag="z_sb")
                            nc.scalar.copy(out=z_sb[:], in_=zt[:])
                            a_sb = mp.tile([P, 512], bf, tag="a_sb")
                            nc.scalar.activation(out=a_sb[:], in_=zt[:],
                                                 func=Abs)
                            for t in range(4):
                                e = 4 * s + t
                                nc.tensor.matmul(
                                    out=pp[:, H * e: H * (e + 1)],
                                    lhsT=z_sb[:, P * t: P * (t + 1)],
                                    rhs=attm_sb[:, :H],
                                    start=True, stop=False)
                                nc.tensor.matmul(
                                    out=pp[:, H * e: H * (e + 1)],
                                    lhsT=a_sb[:, P * t: P * (t + 1)],
                                    rhs=attm_sb[:, H: 2 * H],
                                    start=False, stop=True)

                    # aggregation — den rides as 4 extra rhs columns; one
                    # window-wide exp writes p straight into those columns,
                    # the u*p multiply runs per super-tile so aggregation
                    # matmuls can start as soon as their slice is ready
                    xjw = mp.tile([P, SLOTS, F + H], bf, tag="xjw")
                    nc.scalar.activation(
                        out=xjw[:, :, F:],
                        in_=pp[:].rearrange("p (a b) -> p a b", b=H),
                        func=Exp)
                    nc.vector.tensor_tensor(
                        out=xjw[:, :, 0:F]
                        .rearrange("p t (h c) -> p t h c", h=H),
                        in0=g[:].rearrange("p t (h c) -> p t h c", h=H),
                        in1=xjw[:, :, F:][:, :, :, None]
                        .broadcast_to([P, SLOTS, H, C]),
                        op=mybir.AluOpType.mult)
                    pod = ps_od.tile([P, F + H], f32, tag="pod")
                    for e in range(SLOTS):
                        nc.tensor.matmul(
                            out=pod[:], lhsT=s4[:, e, :],
                            rhs=xjw[:, e, :],
                            start=(e == 0), stop=(e == SLOTS - 1))

                    dn = fp.tile([P, H], f32, tag="dn")
                    nc.vector.tensor_scalar_add(out=dn[:], in0=pod[:, F:],
                                                scalar1=EPS)
                    rd = fp.tile([P, H], f32, tag="rd")
                    nc.vector.reciprocal(out=rd[:], in_=dn[:])
                    fin = fp.tile([P, H, C], f32, tag="fin")
                    nc.vector.tensor_tensor(
                        out=fin[:],
                        in0=pod[:, 0:F].rearrange("p (h c) -> p h c", h=H),
                        in1=rd[:, :, None].broadcast_to([P, H, C]),
                        op=mybir.AluOpType.mult)
                    fin2 = fp.tile([P, F], bf, tag="fin2")
                    nc.vector.tensor_tensor(
                        out=fin2[:],
                        in0=fin[:].rearrange("p h c -> p (h c)"),
                        in1=bias_sb[:], op=mybir.AluOpType.add)
                    nc.sync.dma_start(
                        out=outp[P * w: P * (w + 1), :], in_=fin2[:])

    nc.compile()
    return nc


# ------------------------------------------------------------------- driver


def _prepare(x, edge_index, W_l, W_r, att, bias):
    n_nodes = x.shape[0]
    n_edges = edge_index.shape[1]
    src = np.asarray(edge_index[0], np.int64)
    dst = np.asarray(edge_index[1], np.int64)
    order = np.argsort(dst, kind="stable")
    ssrc = src[order]
    sdst = dst[order]

    eb, nb = _split_cores(sdst, n_nodes, n_edges)
    npad_xl = _round_up(n_nodes, 2048)  # divisible by 4 quarters of 512-mult
    qrows = npad_xl // NQ
    assert qrows <= 32768

    cores = []
    for c in range(NCORES):
        nr_c = nb[c + 1] - nb[c]
        assert nr_c < 32000
        cores.append(_build_core_windows(
            ssrc[eb[c]:eb[c + 1]], sdst[eb[c]:eb[c + 1]], nb[c], nr_c, qrows))

    W = max(cd["wc"] for cd in cores)
    nrx = _round_up(max(W * P, 2048), 2048)

    xb = np.asarray(x, np.float32).astype(BF16)
    xT_np = np.zeros((P, npad_xl), BF16)
    xT_np[:, :n_nodes] = xb.T

    wl_np = np.asarray(W_l, np.float32).astype(BF16)
    wr_np = np.asarray(W_r, np.float32).astype(BF16)
    att_np = np.asarray(att, np.float32)
    attm_np = np.zeros((P, 3 * H), np.float32)
    for h in range(H):
        attm_np[C * h:C * (h + 1), h] = 0.6 * att_np[h]
        attm_np[C * h:C * (h + 1), H + h] = 0.4 * att_np[h]
        attm_np[C * h:C * (h + 1), 2 * H + h] = att_np[h]
    attm_np = attm_np.astype(BF16)
    bias_np = np.asarray(bias, np.float32)
    bias_bc_np = np.tile(bias_np[None, :], (P, 1)).astype(np.float32)

    iota128 = np.arange(P, dtype=np.float32)

    in_maps = []
    for c in range(NCORES):
        cd = cores[c]
        wc = cd["wc"]

        UW = EPW // 16
        WMB = 2 * UW + EPW + 2 * SLOTS
        gidx_np = np.full((W, P, UW), -1 if TRIM_PAD else 0, np.int16)
        # one-hot dst matrix S^T as uint8 bit patterns, viewed fp8
        sT_np = np.zeros((W, P, EPW), np.uint8)
        drel_np = np.full((W, P, SLOTS), DUMMY_COL, np.float32)

        for w in range(wc):
            # uidx is [P, SLOTS] slot-shaped (slot j at [j%128, j//128]);
            # rewrap for the int16 idx plane (value j at [j%16, j//16]).
            uflat = cd["uidx"][w].T.reshape(-1)  # slot order: s*128+p
            gidx_np[w] = _wrap16(uflat, UW)
            dr = cd["dstrel"][w]                 # [P, SLOTS] float
            drel_np[w] = dr
            drf = dr.T.reshape(-1)               # flat slot-major
            sT_np[w] = (iota128[:, None] == drf[None, :]) * np.uint8(FP8_ONE)

        wmeta_np = np.empty((W, P, WMB), np.uint8)
        wmeta_np[:, :, 0:2 * UW] = gidx_np.view(np.uint8)
        wmeta_np[:, :, 2 * UW: 2 * UW + EPW] = sT_np
        wmeta_np[:, :, 2 * UW + EPW:] = \
            drel_np.astype(BF16).view(np.uint8)

        # window-major xr source: column 128*w + j = x[win_nb[w] + j]
        xrT_np = np.zeros((P, nrx), BF16)
        for w in range(wc):
            lo = int(cd["win_nb"][w])
            hi = min(lo + P, n_nodes)
            xrT_np[:, P * w: P * w + (hi - lo)] = xb[lo:hi].T

        in_maps.append({
            "xT": xT_np,
            "xrT": xrT_np,
            "Wl": wl_np,
            "Wr": wr_np,
            "attm": attm_np,
            "bias_bc": bias_bc_np,
            "wmeta": wmeta_np,
        })

    meta = dict(W=W, npad_xl=npad_xl, nrx=nrx, nb=nb,
                n_nodes=n_nodes, bias=bias_np, cores=cores,
                deg=np.bincount(dst, minlength=n_nodes))
    return in_maps, meta


_last_results = None


def kernel(x, edge_index, W_l, W_r, att, bias, _sim=False, _trace=False):
    global _last_results
    in_maps, meta = _prepare(x, edge_index, W_l, W_r, att, bias)
    nc = _build_nc(meta["W"], meta["npad_xl"], meta["nrx"])

    if _sim:
        from concourse.bass_interp import CoreSim
        results = []
        for c in range(NCORES):
            sim = CoreSim(nc, trace=False)
            for k, v in in_maps[c].items():
                sim.tensor(k)[:] = v
            sim.tensor("outp")[:] = 0.0
            sim.simulate()
            results.append({"outp": np.array(sim.tensor("outp"))})
    else:
        from concourse import bass_utils
        r = bass_utils.run_bass_kernel_spmd(
            nc, in_maps, core_ids=list(range(NCORES)), trace=_trace)
        _last_results = r
        results = r.results

    n_nodes = meta["n_nodes"]
    out = np.empty((n_nodes, F), np.float32)
    for c in range(NCORES):
        cd = meta["cores"][c]
        st = results[c]["outp"]
        for w in range(cd["wc"]):
            lo = int(cd["win_nb"][w])
            hi = int(cd["own_end"][w])
            out[lo:hi] = st[P * w: P * w + (hi - lo)]
    out[meta["deg"] == 0] = meta["bias"][None, :]
    return out
